# revision 15
# baseline (speedup 1.0000x reference)
"""MobileMQA3D kernel for 8 Trainium2 NeuronCores.

Reference math (per batch b, with xf = x[b] reshaped [C=512, N=8192]):
    q = (Wq @ xf).T + bq                    # [N, 128]
    k = (Wk @ xf).T + bk                    # [N, 128]
    v = (Wv @ xf).T + bv                    # [N, 128]
    P = softmax(q @ k.T / sqrt(128))        # [N, N]
    o = P @ v                               # [N, 128]
    y = Wo @ tile(o, 4).T + bo + xf         # [C, N]

Algebraic simplifications used:
  * tile(o,4) then Wo  ==  Wo_eff @ o.T with Wo_eff = Wo.reshape(512,4,128).sum(1)
  * bv folds into the output bias: y += Wo_eff @ bv (softmax rows sum to 1)
  * bk drops entirely: k -> k + bk shifts every logit of query q by q.bk,
    a per-query constant over keys, which softmax cancels exactly
  * softmax computed without max subtraction: logits here are ~N(0, 0.2^2)
    (weights scaled 0.02), exp() cannot overflow; identical after
    normalization
  * the 1/sqrt(128) logit scale and the x16 fp8 weight prescale ride the
    activation instruction's free affine (exp(s * SCALE/256))

Sharding: core c handles batch b = c//4 and query chunk s = c%4 (2048
queries).  The host rotates each core's sequence axis so its own query
chunk is always columns 0..2047 (attention is permutation-invariant over
keys), keeping the program SPMD-identical.  k/v are computed for the full
rotated sequence on each core (redundant 4x) - cheaper than AllGather
(~30us per collective on this fabric, serialized).

Schedule notes (what the 205us and 225us earlier cuts got wrong):
  * DMA_DIRECT2D ops serialize on their issuing engine's queue at ~0.6us
    apiece: x ships as fp8 in 8 one-per-1024-column strided DMAs in
    consumption order, the q/k/v weights ride one packed DMA, and xresT +
    half the output stores issue from the GpSimd queue instead of Sync.
  * the PE queue is in-order: PV/dacc for pair p-1 are emitted AFTER the
    S matmuls of pair p, so the PE never sits on an exp semaphore while
    ready S work exists behind it in program order.
  * projections interleave into the attention loop (blocks 0+1), sharing
    kT/v2/ones stationaries across the paired query blocks; blocks 2+3
    are a pure ACT-bound exp stream.
  * block tails (normalize+project+store) are cut into per-engine batched
    pieces and dribbled into the next phase's ACT-bound slack.

Per-core main loop (all streams sized by the ACT exp roof, ~1147ns per
[128,2,512] exp):
    S^T [128k, 2, 512q] = kT_chunk.T @ qT_block  x2 per block  (PE, bf16)
    PT  = exp(S^T * SCALE/256) -> fp8                          (ACT)
    oT  += v2_pair.T @ PT   (DoubleRow fp8)                    (PE)
    dacc += ones2.T @ PT    (DoubleRow fp8, denominator)       (PE)
  per 128-query row: d = (dacc slice).T @ 1/128, DVE reciprocal,
    yT = oT_sub.T @ Wo_effT/16, y = yT * (1/d) + (x.T + bo_eff)  (DVE)
"""

import numpy as np

# ---------------------------------------------------------------- constants
B = 2
C = 512
CO = C // 128          # 4 channel groups
CK = 128               # shared q/k/v head dim
D, H, W = 8, 32, 32
N = D * H * W          # 8192 sequence positions per batch
NCORES = 8
SEQ_SHARDS = NCORES // B          # 4 query chunks per batch
NCH = N // SEQ_SHARDS             # 2048 queries per core
NQB = 512                         # query block (PSUM free dim)
NQBLOCKS = NCH // NQB             # 4
NKC = 128                         # key chunk (matmul stationary width)
NKCHUNKS = N // NKC               # 64
NPAIRS = NKCHUNKS // 2            # 32 key-chunk pairs
SCALE = float(CK) ** -0.5
WSCALE = 16.0                     # fp8 weight prescale (keeps Wq/Wk/Wv
                                  # entries out of the e4m3 subnormal zone)
ESC = SCALE / (WSCALE * WSCALE)   # exp free-affine scale

_cache = {}


def _ensure_axon_hooks_module():
    """run_bass_kernel_spmd(trace=True) under axon imports
    antenv.axon_hooks, which not every image ships.  Register a stub so a
    BASS_TRACE=1 environment degrades to no-trace instead of crashing."""
    import sys

    try:
        import antenv.axon_hooks  # noqa: F401
        return
    except ImportError:
        pass
    import types

    mod = types.ModuleType("antenv.axon_hooks")
    mod._hook = None
    mod.set_axon_ntff_profile_hook = lambda h: setattr(mod, "_hook", h)
    mod.get_axon_ntff_profile_hook = lambda: mod._hook
    sys.modules["antenv.axon_hooks"] = mod
    try:
        import antenv

        antenv.axon_hooks = mod
    except ImportError:
        pass


def _install_drain_patch():
    """This walrus build rejects >1 sem-wait command on the SP Drain that
    Tile emits at kernel tail (one wait per live semaphore).  Split the
    surplus waits across trailing SP nops."""
    import bass_rust
    import concourse.tile as tile_mod
    from concourse.vector_clock import ScopedClock

    if getattr(tile_mod.TileContext, "_ant_drain_split", False):
        return

    def _drain_and_barrier(self, tick_clock, wait_clock):
        nc = self.nc
        drain_inst = nc.sync.drain()
        wait_clock.add_sem_waits(
            drain_inst.ins, ScopedClock({None: tick_clock.global_clock})
        )
        si = drain_inst.ins.sync_info
        waits = list(si.on_wait)
        if len(waits) > 1:
            drain_inst.ins.sync_info = bass_rust.SyncInfo(
                on_wait=waits[:1], on_update=list(si.on_update)
            )
            for i in range(1, len(waits)):
                nop_inst = nc.sync.nop(nofuse=True, hint="drain_wait_split")
                nop_inst.ins.sync_info = bass_rust.SyncInfo(
                    on_wait=waits[i : i + 1], on_update=[]
                )
        nc.all_engine_barrier()
        assert self.sems is not None
        popped = nc._tile_sem_poison_stack.pop()
        assert popped is self._sem_poison
        nc.clear_and_free_semaphores(list(self.sems.allocated().values()))
        nc.all_engine_barrier()

    tile_mod.TileContext._drain_and_barrier = _drain_and_barrier
    tile_mod.TileContext._ant_drain_split = True


def _split_excess_waits(nc, limit=1):
    """This walrus build accepts at most one sem-wait command per engine
    instruction.  Move surplus waits onto same-engine nops inserted right
    before the offending instruction (the engine stalls at each nop, so the
    instruction still starts only after every original wait has cleared)."""
    import bass_rust
    import concourse.mybir as mybir

    n_split = 0
    for fn in nc.m.functions:
        for bb in fn.blocks:
            insts = bb.instructions
            out = []
            dirty = False
            for inst in insts:
                si = inst.sync_info
                waits = list(si.on_wait) if si is not None else []
                if len(waits) > limit:
                    dirty = True
                    keep = waits[-limit:]
                    for j, w in enumerate(waits[:-limit]):
                        nop = mybir.InstNoOp(
                            name=f"{inst.name}_wsplit{j}", ins=[], outs=[]
                        )
                        nop.engine = inst.engine
                        nop.sync_info = bass_rust.SyncInfo(
                            on_wait=[w], on_update=[]
                        )
                        out.append(nop)
                        n_split += 1
                    inst.sync_info = bass_rust.SyncInfo(
                        on_wait=keep, on_update=list(si.on_update)
                    )
                out.append(inst)
            if dirty:
                bb.instructions = out
    return n_split


def build_bass():
    """Build the single-core SPMD bass program (same NEFF on all 8 cores)."""
    import concourse.bass as bass
    import concourse.mybir as mybir
    from concourse.tile import TileContext

    _install_drain_patch()

    f32 = mybir.dt.float32
    bf16 = mybir.dt.bfloat16
    fp8 = mybir.dt.float8e4
    AF = mybir.ActivationFunctionType
    ALU = mybir.AluOpType
    DR = mybir.MatmulPerfMode.DoubleRow

    nc = bass.Bass()

    # ------------------------------------------------------------- DRAM I/O
    x8_d = nc.declare_dram_parameter("x8", [128, CO, N], fp8, isOutput=False)
    w8_d = nc.declare_dram_parameter("w8", [128, 3, CO, CK], fp8, isOutput=False)
    xresT_d = nc.declare_dram_parameter(
        "xresT", [128, NCH // 128, C], f32, isOutput=False
    )
    woeT_d = nc.declare_dram_parameter("woeT", [128, C], bf16, isOutput=False)
    bqs_d = nc.declare_dram_parameter("bqs", [128, 1], f32, isOutput=False)
    out_d = nc.declare_dram_parameter("out", [NCH, C], f32, isOutput=True)

    with TileContext(nc) as tc:
        singles = tc.alloc_tile_pool(name="singles", bufs=1)
        persist = tc.alloc_tile_pool(name="persist", bufs=1)
        pt_pool = tc.alloc_tile_pool(name="pt_pool", bufs=6)
        small_sb = tc.alloc_tile_pool(name="small_sb", bufs=4)
        ysb_pool = tc.alloc_tile_pool(name="ysb_pool", bufs=4)
        # PSUM budget (8 banks): sp 2x2 + oT 2x1 + dacc 2x1 = 8.
        # All transient [128,<=1024]-f32 psum needs (q/k/v projections, the
        # per-128-row denominator + output-projection tiles) share the "sp"
        # tag rotation.
        ps_pair = tc.alloc_tile_pool(name="ps_pair", bufs=2, space="PSUM")
        ps_acc = tc.alloc_tile_pool(name="ps_acc", bufs=2, space="PSUM")

        # ------------------------------------------------ weight/input loads
        # One packed DMA for the three projection weights; x8 in 8 strided
        # 512KB DMAs in consumption order.  Sync-queue DMAs serialize at
        # ~0.6us apiece, so ordering here is the kernel's warm-up path.
        w8_sb = singles.tile([128, 3, CO, CK], fp8)
        bqs_sb = singles.tile([128, 1], f32)
        woeT_sb = singles.tile([128, C], bf16)
        ones2 = singles.tile([128, 2, 128], fp8)
        inv128 = singles.tile([128, 1], bf16)
        x8_sb = persist.tile([128, CO, N], fp8)
        xresT_sb = persist.tile([128, NCH // 128, C], f32)

        nc.sync.dma_start(out=w8_sb, in_=w8_d[:])
        nc.sync.dma_start(out=bqs_sb, in_=bqs_d[:])
        for nb in range(8):
            sl = slice(nb * (N // 8), (nb + 1) * (N // 8))
            nc.sync.dma_start(out=x8_sb[:, :, sl], in_=x8_d[:, :, sl])
        nc.sync.dma_start(out=woeT_sb, in_=woeT_d[:])
        # xresT is needed only from the first block tail (~halfway in).
        # Keep it on the Sync queue BEHIND the x8 slices: issuing it in
        # parallel from another queue makes its 4MB compete with the
        # critical x8 transfers for HBM and delays the first exp by ~15us.
        for nb in range(4):
            sl = slice(nb * 4, (nb + 1) * 4)
            nc.sync.dma_start(out=xresT_sb[:, sl, :], in_=xresT_d[:, sl, :])
        nc.vector.memset(ones2, 1.0)
        nc.vector.memset(inv128, 1.0 / 128.0)

        wq8 = w8_sb[:, 0]
        wk8 = w8_sb[:, 1]
        wv8 = w8_sb[:, 2]

        qT_sb = persist.tile([128, NCH], bf16)
        kT_sb = persist.tile([128, N], bf16)
        v2_sb = persist.tile([128, NPAIRS, 2, CK], fp8)

        def q_proj(nb):
            """qT block nb = 16*(Wq @ x + bq), bf16.  DoubleRow fp8."""
            qps = ps_pair.tile([128, 2, NQB], f32, tag="sp", name="qps")
            for cp in range(CO // 2):
                nc.tensor.matmul(
                    qps[:, 0, :],
                    lhsT=wq8[:, 2 * cp : 2 * cp + 2, :],
                    rhs=x8_sb[:, 2 * cp : 2 * cp + 2, nb * NQB : (nb + 1) * NQB],
                    start=(cp == 0),
                    stop=(cp == CO // 2 - 1),
                    perf_mode=DR,
                )
            nc.vector.tensor_scalar_add(
                qT_sb[:, nb * NQB : (nb + 1) * NQB], qps[:, 0, :], bqs_sb[:, 0:1]
            )

        def k_proj_into(sp, j):
            """kT for key block j, computed in the scratch half of an S
            tile about to be overwritten (keeps the 2-deep PSUM rotation at
            exactly two allocations per pair)."""
            bsl = slice(j * NQB, (j + 1) * NQB)
            for cp in range(CO // 2):
                nc.tensor.matmul(
                    sp[:, 0, :],
                    lhsT=wk8[:, 2 * cp : 2 * cp + 2, :],
                    rhs=x8_sb[:, 2 * cp : 2 * cp + 2, bsl],
                    start=(cp == 0),
                    stop=(cp == CO // 2 - 1),
                    perf_mode=DR,
                )
            nc.vector.tensor_copy(out=kT_sb[:, bsl], in_=sp[:, 0, :])

        def v_chunks_into(sp, j, half):
            """v2 for key-chunk pair 2j+half (2 chunks of 128): plain fp8
            matmuls, stationary x8 chunk (128-col FWL load), moving wv8."""
            for c in range(2):
                kc = 4 * j + 2 * half + c
                for ci in range(CO):
                    nc.tensor.matmul(
                        sp[:, 0, c * CK : (c + 1) * CK],
                        lhsT=x8_sb[:, ci, kc * NKC : (kc + 1) * NKC],
                        rhs=wv8[:, ci, :],
                        start=(ci == 0),
                        stop=(ci == CO - 1),
                    )
            nc.vector.tensor_copy(
                out=v2_sb[:, 2 * j + half, :, :], in_=sp[:, 0, 0 : 2 * CK]
            )

        def emit_pair(p, blocks, proj=(None, None)):
            """S matmuls + exp for key-chunk pair p; kT stationary serves
            both query blocks.  proj closures run first, inside the pair's
            S tiles (the S matmuls overwrite them after the DVE copy-out).
            Returns the fp8 PT tiles."""
            sps = []
            for bi, b in enumerate(blocks):
                sp = ps_pair.tile([128, 2, NQB], f32, tag="sp", name=f"sp{b}")
                if proj[bi] is not None:
                    proj[bi](sp)
                sps.append(sp)
            for h in range(2):
                kc = 2 * p + h
                for bi, b in enumerate(blocks):
                    nc.tensor.matmul(
                        sps[bi][:, h, :],
                        lhsT=kT_sb[:, kc * NKC : (kc + 1) * NKC],
                        rhs=qT_sb[:, b * NQB : (b + 1) * NQB],
                        start=True,
                        stop=True,
                    )
            pts = []
            for bi, b in enumerate(blocks):
                pt = pt_pool.tile([128, 2, NQB], fp8, tag="pt", name=f"pt{b}")
                nc.scalar.activation(out=pt, in_=sps[bi], func=AF.Exp, scale=ESC)
                pts.append(pt)
            return pts

        def pv_dacc(p, pts, oTs, daccs):
            """Value-accumulate + denominator for pair p (emitted one pair
            behind the S/exp stream so the in-order PE queue never stalls
            on an exp semaphore with S work ready behind it)."""
            for bi in range(len(pts)):
                nc.tensor.matmul(
                    oTs[bi],
                    lhsT=v2_sb[:, p, :, :],
                    rhs=pts[bi],
                    start=(p == 0),
                    stop=(p == NPAIRS - 1),
                    perf_mode=DR,
                )
            for bi in range(len(pts)):
                nc.tensor.matmul(
                    daccs[bi],
                    lhsT=ones2,
                    rhs=pts[bi],
                    start=(p == 0),
                    stop=(p == NPAIRS - 1),
                    perf_mode=DR,
                )

        def block_tail_pieces(specs, queues, acc=False):
            """Emit-closures for normalize+project+residual+store of the
            given (block, oT_ps, dacc_ps) specs.  The two blocks' subs are
            zipped so their independent [denominator-mm -> recip ->
            output-mm -> scale-add -> store] chains hide each other's
            latency inside the 2-deep "sp" PSUM rotation.  Stores rotate
            over the given DMA-issue queues."""
            pieces = []
            state = {}

            def copies(b, oT_ps, dacc_ps):
                def run():
                    oT_sb = small_sb.tile(
                        [128, NQB], bf16, tag="oT", bufs=2, name="oT_sb"
                    )
                    nc.vector.tensor_copy(out=oT_sb, in_=oT_ps)
                    # bf16 denominators: values ~N, 0.4% rounding is far
                    # inside the fp8 noise floor, and bf16 weights get the
                    # fast LDWEIGHTS path for the transpose matmul below
                    dsb = small_sb.tile(
                        [128, NQB], bf16, tag="dsb", bufs=2, name="dsb"
                    )
                    nc.vector.tensor_copy(out=dsb, in_=dacc_ps)
                    state[b] = (oT_sb, dsb)

                return run

            def sub_piece(b, sub, eng):
                def run():
                    oT_sb, dsb = state[b]
                    ssl = slice(sub * 128, (sub + 1) * 128)
                    if acc:
                        # end-of-kernel: the attention accumulator banks
                        # are free, use them instead of the S rotation
                        dts = ps_acc.tile(
                            [128, NQB], f32, tag="dacc", name="dts"
                        )
                        yts = ps_acc.tile([128, NQB], f32, tag="oT", name="yts")
                        d_ap, y_ap = dts[:, 0:1], yts
                    else:
                        dyt = ps_pair.tile(
                            [128, 2, NQB], f32, tag="sp", name="dyt"
                        )
                        d_ap, y_ap = dyt[:, 0, 0:1], dyt[:, 1, :]
                    # delta is identical in every dacc row; summing a
                    # 128-column slice over partitions against 1/128
                    # transposes it to [128, 1]
                    nc.tensor.matmul(
                        d_ap, lhsT=dsb[:, ssl], rhs=inv128,
                        start=True, stop=True,
                    )
                    dr = small_sb.tile([128, 1], f32, tag="dr", bufs=8, name="dr")
                    nc.vector.reciprocal(out=dr, in_=d_ap)
                    nc.tensor.matmul(
                        y_ap, lhsT=oT_sb[:, ssl], rhs=woeT_sb,
                        start=True, stop=True,
                    )
                    y_sb = ysb_pool.tile([128, C], f32, tag="y", name="y_sb")
                    nq_row = b * (NQB // 128) + sub
                    # y = y_ps / delta + (x.T + bo_eff)   (one DVE pass)
                    nc.vector.scalar_tensor_tensor(
                        y_sb,
                        y_ap,
                        dr[:, 0:1],
                        xresT_sb[:, nq_row, :],
                        ALU.mult,
                        ALU.add,
                    )
                    eng.dma_start(
                        out=out_d[nq_row * 128 : (nq_row + 1) * 128, :], in_=y_sb
                    )

                return run

            for b, oT_ps, dacc_ps in specs:
                pieces.append(copies(b, oT_ps, dacc_ps))
            qd = 0
            for sub in range(NQB // 128):
                for b, _, _ in specs:
                    pieces.append(sub_piece(b, sub, queues[qd % len(queues)]))
                    qd += 1
            return pieces

        # ---------------------------------------- merged proj+attention loop
        # Blocks 0+1 run while the kT/v2 projections stream in.
        oT0 = ps_acc.tile([128, NQB], f32, tag="oT", name="oT0")
        oT1 = ps_acc.tile([128, NQB], f32, tag="oT", name="oT1")
        dacc0 = ps_acc.tile([128, NQB], f32, tag="dacc", name="dacc0")
        dacc1 = ps_acc.tile([128, NQB], f32, tag="dacc", name="dacc1")
        oTsA, daccsA = (oT0, oT1), (dacc0, dacc1)

        # Warm the PE (HAM un-throttles after ~3.4us of sustained matmul)
        # on weights-only junk while the first x8 slice is still in
        # flight, so the real head runs at 2.4 GHz.
        warm = ps_pair.tile([128, 2, NQB], f32, tag="sp", name="warm")
        for _ in range(10):
            nc.tensor.matmul(
                warm[:, 0, :], lhsT=w8_sb[:, 0, 0, :], rhs=w8_sb[:, 0],
                start=True, stop=True,
            )

        # Phase A needs only qT blocks 0,1 (blocks 2,3 are projected once
        # the pipeline is rolling).  Each pair carries its own projection
        # slice inside its S tiles: kT for block j rides pair 2j-1 (pair 0
        # for j=0), v2 for pair p rides pair p, always one pair ahead of
        # the value matmuls, which lag the exp stream by one pair.
        q_proj(0)
        q_proj(1)
        pend = None
        for p in range(NPAIRS):
            j, half = p // 2, p % 2
            if p == 0:
                proj = (
                    lambda sp: k_proj_into(sp, 0),
                    lambda sp: v_chunks_into(sp, 0, 0),
                )
            elif half == 1:
                j2 = (p + 1) // 2
                proj = (
                    lambda sp, j=j, half=half: v_chunks_into(sp, j, half),
                    (lambda sp, j2=j2: k_proj_into(sp, j2))
                    if j2 < NKCHUNKS // 4
                    else None,
                )
            else:
                proj = (
                    lambda sp, j=j: v_chunks_into(sp, j, 0),
                    None,
                )
            pts = emit_pair(p, (0, 1), proj)
            if pend is not None:
                pv_dacc(pend[0], pend[1], oTsA, daccsA)
            pend = (p, pts)
            if p == 2:
                q_proj(2)
            elif p == 3:
                q_proj(3)
        pv_dacc(pend[0], pend[1], oTsA, daccsA)

        # Blocks 2+3: pure attention (ACT-bound), kT/v2 already resident.
        # Blocks 0+1's tails dribble into the PE/DVE slack of this phase.
        oT2 = ps_acc.tile([128, NQB], f32, tag="oT", name="oT2")
        oT3 = ps_acc.tile([128, NQB], f32, tag="oT", name="oT3")
        dacc2 = ps_acc.tile([128, NQB], f32, tag="dacc", name="dacc2")
        dacc3 = ps_acc.tile([128, NQB], f32, tag="dacc", name="dacc3")
        oTsB, daccsB = (oT2, oT3), (dacc2, dacc3)

        tailsA = block_tail_pieces(
            [(0, oT0, dacc0), (1, oT1, dacc1)], [nc.sync, nc.gpsimd]
        )
        pend = None
        ti = 0
        for p in range(NPAIRS):
            pts = emit_pair(p, (2, 3))
            if pend is not None:
                pv_dacc(pend[0], pend[1], oTsB, daccsB)
            pend = (p, pts)
            if p >= 1 and ti < len(tailsA):
                tailsA[ti]()
                ti += 1
        pv_dacc(pend[0], pend[1], oTsB, daccsB)
        while ti < len(tailsA):
            tailsA[ti]()
            ti += 1

        # End tails: nothing left to hide behind.  Use the freed attention
        # accumulator banks instead of the S rotation and spread the store
        # issues over queues whose engines are idle by now.
        for piece in block_tail_pieces(
            [(2, oT2, dacc2), (3, oT3, dacc3)],
            [nc.sync, nc.scalar, nc.gpsimd],
            acc=True,
        ):
            piece()

        for pool in (
            ps_acc,
            ps_pair,
            ysb_pool,
            small_sb,
            pt_pool,
            persist,
            singles,
        ):
            pool.release()

    _split_excess_waits(nc)
    return nc


def _prep_weights(Wq, bq, Wk, bk, Wv, bv, Wo, bo):
    import ml_dtypes

    bf = ml_dtypes.bfloat16
    f8 = ml_dtypes.float8_e4m3fn

    def wT8(Wm):  # [o, C] -> lhsT layout [ci, cio, o], fp8, x16 prescale
        return np.ascontiguousarray(
            (Wm * WSCALE).T.reshape(CO, 128, -1).transpose(1, 0, 2)
        ).astype(f8)

    Wo_eff = Wo.reshape(C, CO, CK).sum(axis=1)            # [C, CK]
    bo_eff = bo + Wo_eff @ bv                             # [C]
    w8 = np.ascontiguousarray(
        np.stack([wT8(Wq), wT8(Wk), wT8(Wv)], axis=1)
    )                                                      # [128, 3, CO, CK]
    return {
        "w8": w8,
        # oT accumulates 16*o; divide back out through the output projection
        "woeT": np.ascontiguousarray(Wo_eff.T / WSCALE).astype(bf),  # [CK, C]
        "bqs": (bq * WSCALE).reshape(128, 1).astype(np.float32),
    }, bo_eff


def kernel(x, Wq, bq, Wk, bk, Wv, bv, Wo, bo):
    import ml_dtypes

    _ensure_axon_hooks_module()
    from concourse.bass_utils import run_bass_kernel_spmd

    f8 = ml_dtypes.float8_e4m3fn
    x = np.asarray(x, dtype=np.float32)
    wmaps, bo_eff = _prep_weights(
        np.asarray(Wq, np.float32),
        np.asarray(bq, np.float32),
        np.asarray(Wk, np.float32),
        np.asarray(bk, np.float32),
        np.asarray(Wv, np.float32),
        np.asarray(bv, np.float32),
        np.asarray(Wo, np.float32),
        np.asarray(bo, np.float32),
    )

    xf = x.reshape(B, C, N)
    x8_b = []
    for b in range(B):
        x8_b.append(
            np.ascontiguousarray(
                xf[b].reshape(CO, 128, N).transpose(1, 0, 2)
            ).astype(f8)
        )
    in_maps = []
    for core in range(NCORES):
        b, s = divmod(core, SEQ_SHARDS)
        # rotate the sequence axis so this core's query chunk sits at 0
        x8 = np.roll(x8_b[b], -s * NCH, axis=2) if s else x8_b[b]
        xchunkT = xf[b][:, s * NCH : (s + 1) * NCH].T  # [NCH, C]
        xresT = np.ascontiguousarray(
            (xchunkT + bo_eff[None, :])
            .reshape(NCH // 128, 128, C)
            .transpose(1, 0, 2)
        ).astype(np.float32)
        in_maps.append({"x8": x8, "xresT": xresT, **wmaps})

    if "nc" not in _cache:
        _cache["nc"] = build_bass()
    res = run_bass_kernel_spmd(_cache["nc"], in_maps, list(range(NCORES)))
    _cache["last_results"] = res

    y = np.empty((B, C, N), dtype=np.float32)
    for core in range(NCORES):
        b, s = divmod(core, SEQ_SHARDS)
        y[b][:, s * NCH : (s + 1) * NCH] = res.results[core]["out"].T
    return y.reshape(B, C, D, H, W)


# revision 18
# speedup vs baseline: 1.0169x; 1.0169x over previous
"""MobileMQA3D kernel for 8 Trainium2 NeuronCores.

Reference math (per batch b, with xf = x[b] reshaped [C=512, N=8192]):
    q = (Wq @ xf).T + bq                    # [N, 128]
    k = (Wk @ xf).T + bk                    # [N, 128]
    v = (Wv @ xf).T + bv                    # [N, 128]
    P = softmax(q @ k.T / sqrt(128))        # [N, N]
    o = P @ v                               # [N, 128]
    y = Wo @ tile(o, 4).T + bo + xf         # [C, N]

Algebraic simplifications used:
  * tile(o,4) then Wo  ==  Wo_eff @ o.T with Wo_eff = Wo.reshape(512,4,128).sum(1)
  * bv folds into the output bias: y += Wo_eff @ bv (softmax rows sum to 1)
  * bk drops entirely: k -> k + bk shifts every logit of query q by q.bk,
    a per-query constant over keys, which softmax cancels exactly
  * softmax computed without max subtraction: logits here are ~N(0, 0.2^2)
    (weights scaled 0.02), exp() cannot overflow; identical after
    normalization
  * the 1/sqrt(128) logit scale and the x16 fp8 weight prescale ride the
    activation instruction's free affine (exp(s * SCALE/256))

Sharding: core c handles batch b = c//4 and query chunk s = c%4 (2048
queries).  The host rotates each core's sequence axis so its own query
chunk is always columns 0..2047 (attention is permutation-invariant over
keys), keeping the program SPMD-identical.  k/v are computed for the full
rotated sequence on each core (redundant 4x) - cheaper than AllGather
(~30us per collective on this fabric, serialized).

Schedule notes (what the 205us and 225us earlier cuts got wrong):
  * DMA_DIRECT2D ops serialize on their issuing engine's queue at ~0.6us
    apiece: x ships as fp8 in 8 one-per-1024-column strided DMAs in
    consumption order, the q/k/v weights ride one packed DMA, and xresT +
    half the output stores issue from the GpSimd queue instead of Sync.
  * the PE queue is in-order: PV/dacc for pair p-1 are emitted AFTER the
    S matmuls of pair p, so the PE never sits on an exp semaphore while
    ready S work exists behind it in program order.
  * projections interleave into the attention loop (blocks 0+1), sharing
    kT/v2/ones stationaries across the paired query blocks; blocks 2+3
    are a pure ACT-bound exp stream.
  * block tails (normalize+project+store) are cut into per-engine batched
    pieces and dribbled into the next phase's ACT-bound slack.

Per-core main loop (all streams sized by the ACT exp roof, ~1147ns per
[128,2,512] exp):
    S^T [128k, 2, 512q] = kT_chunk.T @ qT_block  x2 per block  (PE, bf16)
    PT  = exp(S^T * SCALE/256) -> fp8                          (ACT)
    oT  += v2_pair.T @ PT   (DoubleRow fp8)                    (PE)
    dacc += ones2.T @ PT    (DoubleRow fp8, denominator)       (PE)
  per 128-query row: d = (dacc slice).T @ 1/128, DVE reciprocal,
    yT = oT_sub.T @ Wo_effT/16, y = yT * (1/d) + (x.T + bo_eff)  (DVE)
"""

import numpy as np

# ---------------------------------------------------------------- constants
B = 2
C = 512
CO = C // 128          # 4 channel groups
CK = 128               # shared q/k/v head dim
D, H, W = 8, 32, 32
N = D * H * W          # 8192 sequence positions per batch
NCORES = 8
SEQ_SHARDS = NCORES // B          # 4 query chunks per batch
NCH = N // SEQ_SHARDS             # 2048 queries per core
NQB = 512                         # query block (PSUM free dim)
NQBLOCKS = NCH // NQB             # 4
NKC = 128                         # key chunk (matmul stationary width)
NKCHUNKS = N // NKC               # 64
NPAIRS = NKCHUNKS // 2            # 32 key-chunk pairs
SCALE = float(CK) ** -0.5
WSCALE = 16.0                     # fp8 weight prescale (keeps Wq/Wk/Wv
                                  # entries out of the e4m3 subnormal zone)
ESC = SCALE / (WSCALE * WSCALE)   # exp free-affine scale

_cache = {}


def _ensure_axon_hooks_module():
    """run_bass_kernel_spmd(trace=True) under axon imports
    antenv.axon_hooks, which not every image ships.  Register a stub so a
    BASS_TRACE=1 environment degrades to no-trace instead of crashing."""
    import sys

    try:
        import antenv.axon_hooks  # noqa: F401
        return
    except ImportError:
        pass
    import types

    mod = types.ModuleType("antenv.axon_hooks")
    mod._hook = None
    mod.set_axon_ntff_profile_hook = lambda h: setattr(mod, "_hook", h)
    mod.get_axon_ntff_profile_hook = lambda: mod._hook
    sys.modules["antenv.axon_hooks"] = mod
    try:
        import antenv

        antenv.axon_hooks = mod
    except ImportError:
        pass


def _install_drain_patch():
    """This walrus build rejects >1 sem-wait command on the SP Drain that
    Tile emits at kernel tail (one wait per live semaphore).  Split the
    surplus waits across trailing SP nops."""
    import bass_rust
    import concourse.tile as tile_mod
    from concourse.vector_clock import ScopedClock

    if getattr(tile_mod.TileContext, "_ant_drain_split", False):
        return

    def _drain_and_barrier(self, tick_clock, wait_clock):
        nc = self.nc
        drain_inst = nc.sync.drain()
        wait_clock.add_sem_waits(
            drain_inst.ins, ScopedClock({None: tick_clock.global_clock})
        )
        si = drain_inst.ins.sync_info
        waits = list(si.on_wait)
        if len(waits) > 1:
            drain_inst.ins.sync_info = bass_rust.SyncInfo(
                on_wait=waits[:1], on_update=list(si.on_update)
            )
            for i in range(1, len(waits)):
                nop_inst = nc.sync.nop(nofuse=True, hint="drain_wait_split")
                nop_inst.ins.sync_info = bass_rust.SyncInfo(
                    on_wait=waits[i : i + 1], on_update=[]
                )
        nc.all_engine_barrier()
        assert self.sems is not None
        popped = nc._tile_sem_poison_stack.pop()
        assert popped is self._sem_poison
        nc.clear_and_free_semaphores(list(self.sems.allocated().values()))
        nc.all_engine_barrier()

    tile_mod.TileContext._drain_and_barrier = _drain_and_barrier
    tile_mod.TileContext._ant_drain_split = True


def _split_excess_waits(nc, limit=1):
    """This walrus build accepts at most one sem-wait command per engine
    instruction.  Move surplus waits onto same-engine nops inserted right
    before the offending instruction (the engine stalls at each nop, so the
    instruction still starts only after every original wait has cleared)."""
    import bass_rust
    import concourse.mybir as mybir

    n_split = 0
    for fn in nc.m.functions:
        for bb in fn.blocks:
            insts = bb.instructions
            out = []
            dirty = False
            for inst in insts:
                si = inst.sync_info
                waits = list(si.on_wait) if si is not None else []
                if len(waits) > limit:
                    dirty = True
                    keep = waits[-limit:]
                    for j, w in enumerate(waits[:-limit]):
                        nop = mybir.InstNoOp(
                            name=f"{inst.name}_wsplit{j}", ins=[], outs=[]
                        )
                        nop.engine = inst.engine
                        nop.sync_info = bass_rust.SyncInfo(
                            on_wait=[w], on_update=[]
                        )
                        out.append(nop)
                        n_split += 1
                    inst.sync_info = bass_rust.SyncInfo(
                        on_wait=keep, on_update=list(si.on_update)
                    )
                out.append(inst)
            if dirty:
                bb.instructions = out
    return n_split


def build_bass():
    """Build the single-core SPMD bass program (same NEFF on all 8 cores)."""
    import concourse.bass as bass
    import concourse.mybir as mybir
    from concourse.tile import TileContext

    _install_drain_patch()

    f32 = mybir.dt.float32
    bf16 = mybir.dt.bfloat16
    fp8 = mybir.dt.float8e4
    AF = mybir.ActivationFunctionType
    ALU = mybir.AluOpType
    DR = mybir.MatmulPerfMode.DoubleRow

    nc = bass.Bass()

    # ------------------------------------------------------------- DRAM I/O
    x8_d = nc.declare_dram_parameter("x8", [128, CO, N], fp8, isOutput=False)
    w8_d = nc.declare_dram_parameter("w8", [128, 3, CO, CK], fp8, isOutput=False)
    xresT_d = nc.declare_dram_parameter(
        "xresT", [128, NCH // 128, C], f32, isOutput=False
    )
    woeT_d = nc.declare_dram_parameter("woeT", [128, C], bf16, isOutput=False)
    bqs_d = nc.declare_dram_parameter("bqs", [128, 1], f32, isOutput=False)
    out_d = nc.declare_dram_parameter("out", [NCH, C], f32, isOutput=True)

    with TileContext(nc) as tc:
        singles = tc.alloc_tile_pool(name="singles", bufs=1)
        persist = tc.alloc_tile_pool(name="persist", bufs=1)
        pt_pool = tc.alloc_tile_pool(name="pt_pool", bufs=6)
        small_sb = tc.alloc_tile_pool(name="small_sb", bufs=4)
        ysb_pool = tc.alloc_tile_pool(name="ysb_pool", bufs=4)
        # PSUM budget (8 banks): sp 2x2 + oT 2x1 + dacc 2x1 = 8.
        # All transient [128,<=1024]-f32 psum needs (q/k/v projections, the
        # per-128-row denominator + output-projection tiles) share the "sp"
        # tag rotation.
        ps_pair = tc.alloc_tile_pool(name="ps_pair", bufs=2, space="PSUM")
        ps_acc = tc.alloc_tile_pool(name="ps_acc", bufs=2, space="PSUM")

        # ------------------------------------------------ weight/input loads
        # One packed DMA for the three projection weights; x8 in 8 strided
        # 512KB DMAs in consumption order.  Sync-queue DMAs serialize at
        # ~0.6us apiece, so ordering here is the kernel's warm-up path.
        w8_sb = singles.tile([128, 3, CO, CK], fp8)
        bqs_sb = singles.tile([128, 1], f32)
        woeT_sb = singles.tile([128, C], bf16)
        ones2 = singles.tile([128, 2, 128], fp8)
        inv128 = singles.tile([128, 1], bf16)
        x8_sb = persist.tile([128, CO, N], fp8)
        xresT_sb = persist.tile([128, NCH // 128, C], f32)

        nc.sync.dma_start(out=w8_sb, in_=w8_d[:])
        nc.sync.dma_start(out=bqs_sb, in_=bqs_d[:])
        # first 512 columns ride alone: they unblock qT block 0, kT block 0
        # and v2 pair 0 - the entire critical path to the first exp
        nc.sync.dma_start(out=x8_sb[:, :, 0:512], in_=x8_d[:, :, 0:512])
        nc.sync.dma_start(out=x8_sb[:, :, 512:1024], in_=x8_d[:, :, 512:1024])
        for nb in range(1, 8):
            sl = slice(nb * (N // 8), (nb + 1) * (N // 8))
            nc.sync.dma_start(out=x8_sb[:, :, sl], in_=x8_d[:, :, sl])
        nc.sync.dma_start(out=woeT_sb, in_=woeT_d[:])
        # xresT is needed only from the first block tail (~halfway in).
        # Keep it on the Sync queue BEHIND the x8 slices: issuing it in
        # parallel from another queue makes its 4MB compete with the
        # critical x8 transfers for HBM and delays the first exp by ~15us.
        for nb in range(4):
            sl = slice(nb * 4, (nb + 1) * 4)
            nc.sync.dma_start(out=xresT_sb[:, sl, :], in_=xresT_d[:, sl, :])
        nc.vector.memset(ones2, 1.0)
        nc.vector.memset(inv128, 1.0 / 128.0)

        wq8 = w8_sb[:, 0]
        wk8 = w8_sb[:, 1]
        wv8 = w8_sb[:, 2]

        qT_sb = persist.tile([128, NCH], bf16)
        kT_sb = persist.tile([128, N], bf16)
        v2_sb = persist.tile([128, NPAIRS, 2, CK], fp8)

        def q_proj(nb):
            """qT block nb = 16*(Wq @ x + bq), bf16.  DoubleRow fp8."""
            qps = ps_pair.tile([128, 2, NQB], f32, tag="sp", name="qps")
            q_proj_into(qps, nb)

        def q_proj_into(sp, nb):
            for cp in range(CO // 2):
                nc.tensor.matmul(
                    sp[:, 1, :],
                    lhsT=wq8[:, 2 * cp : 2 * cp + 2, :],
                    rhs=x8_sb[:, 2 * cp : 2 * cp + 2, nb * NQB : (nb + 1) * NQB],
                    start=(cp == 0),
                    stop=(cp == CO // 2 - 1),
                    perf_mode=DR,
                )
            nc.vector.tensor_scalar_add(
                qT_sb[:, nb * NQB : (nb + 1) * NQB], sp[:, 1, :], bqs_sb[:, 0:1]
            )

        def k_proj_into(sp, j):
            """kT for key block j, computed in the second half of an S tile
            about to be overwritten (keeps the 2-deep PSUM rotation at
            exactly two allocations per pair).  The S matmul into [:,0,:]
            carries no dependency on the copy-out, so the only serial
            tail is the h=1 matmul waiting on the DVE copy."""
            bsl = slice(j * NQB, (j + 1) * NQB)
            for cp in range(CO // 2):
                nc.tensor.matmul(
                    sp[:, 1, :],
                    lhsT=wk8[:, 2 * cp : 2 * cp + 2, :],
                    rhs=x8_sb[:, 2 * cp : 2 * cp + 2, bsl],
                    start=(cp == 0),
                    stop=(cp == CO // 2 - 1),
                    perf_mode=DR,
                )
            nc.vector.tensor_copy(out=kT_sb[:, bsl], in_=sp[:, 1, :])

        def v_chunks_into(sp, j, half):
            """v2 for key-chunk pair 2j+half (2 chunks of 128): plain fp8
            matmuls, stationary x8 chunk (128-col FWL load), moving wv8."""
            for c in range(2):
                kc = 4 * j + 2 * half + c
                for ci in range(CO):
                    nc.tensor.matmul(
                        sp[:, 1, c * CK : (c + 1) * CK],
                        lhsT=x8_sb[:, ci, kc * NKC : (kc + 1) * NKC],
                        rhs=wv8[:, ci, :],
                        start=(ci == 0),
                        stop=(ci == CO - 1),
                    )
            nc.vector.tensor_copy(
                out=v2_sb[:, 2 * j + half, :, :], in_=sp[:, 1, 0 : 2 * CK]
            )

        def emit_pair(p, blocks, proj=(None, None)):
            """S matmuls + exp for key-chunk pair p; kT stationary serves
            both query blocks.  proj closures run first, inside the [:,1,:]
            half of the pair's S tiles (the h=0 S matmul is then free of
            any copy-out dependency).  Returns the fp8 PT tiles."""
            sps = []
            for bi, b in enumerate(blocks):
                sp = ps_pair.tile([128, 2, NQB], f32, tag="sp", name=f"sp{b}")
                if proj[bi] is not None:
                    proj[bi](sp)
                sps.append(sp)
            for h in range(2):
                kc = 2 * p + h
                for bi, b in enumerate(blocks):
                    nc.tensor.matmul(
                        sps[bi][:, h, :],
                        lhsT=kT_sb[:, kc * NKC : (kc + 1) * NKC],
                        rhs=qT_sb[:, b * NQB : (b + 1) * NQB],
                        start=True,
                        stop=True,
                    )
            pts = []
            for bi, b in enumerate(blocks):
                pt = pt_pool.tile([128, 2, NQB], fp8, tag="pt", name=f"pt{b}")
                nc.scalar.activation(out=pt, in_=sps[bi], func=AF.Exp, scale=ESC)
                pts.append(pt)
            return pts

        def pv_dacc(p, pts, oTs, daccs):
            """Value-accumulate + denominator for pair p (emitted one pair
            behind the S/exp stream so the in-order PE queue never stalls
            on an exp semaphore with S work ready behind it)."""
            for bi in range(len(pts)):
                nc.tensor.matmul(
                    oTs[bi],
                    lhsT=v2_sb[:, p, :, :],
                    rhs=pts[bi],
                    start=(p == 0),
                    stop=(p == NPAIRS - 1),
                    perf_mode=DR,
                )
            for bi in range(len(pts)):
                nc.tensor.matmul(
                    daccs[bi],
                    lhsT=ones2,
                    rhs=pts[bi],
                    start=(p == 0),
                    stop=(p == NPAIRS - 1),
                    perf_mode=DR,
                )

        def block_tail_pieces(specs, queues, acc=False):
            """Emit-closures for normalize+project+residual+store of the
            given (block, oT_ps, dacc_ps) specs.  The two blocks' subs are
            zipped so their independent [denominator-mm -> recip ->
            output-mm -> scale-add -> store] chains hide each other's
            latency inside the 2-deep "sp" PSUM rotation.  Stores rotate
            over the given DMA-issue queues."""
            pieces = []
            state = {}

            def copies(b, oT_ps, dacc_ps):
                def run():
                    oT_sb = small_sb.tile(
                        [128, NQB], bf16, tag="oT", bufs=2, name="oT_sb"
                    )
                    nc.vector.tensor_copy(out=oT_sb, in_=oT_ps)
                    # bf16 denominators: values ~N, 0.4% rounding is far
                    # inside the fp8 noise floor, and bf16 weights get the
                    # fast LDWEIGHTS path for the transpose matmul below
                    dsb = small_sb.tile(
                        [128, NQB], bf16, tag="dsb", bufs=2, name="dsb"
                    )
                    nc.vector.tensor_copy(out=dsb, in_=dacc_ps)
                    state[b] = (oT_sb, dsb)

                return run

            def sub_piece(b, sub, eng):
                def run():
                    oT_sb, dsb = state[b]
                    ssl = slice(sub * 128, (sub + 1) * 128)
                    if acc:
                        # end-of-kernel: the attention accumulator banks
                        # are free, use them instead of the S rotation
                        dts = ps_acc.tile(
                            [128, NQB], f32, tag="dacc", name="dts"
                        )
                        yts = ps_acc.tile([128, NQB], f32, tag="oT", name="yts")
                        d_ap, y_ap = dts[:, 0:1], yts
                    else:
                        dyt = ps_pair.tile(
                            [128, 2, NQB], f32, tag="sp", name="dyt"
                        )
                        d_ap, y_ap = dyt[:, 0, 0:1], dyt[:, 1, :]
                    # delta is identical in every dacc row; summing a
                    # 128-column slice over partitions against 1/128
                    # transposes it to [128, 1]
                    nc.tensor.matmul(
                        d_ap, lhsT=dsb[:, ssl], rhs=inv128,
                        start=True, stop=True,
                    )
                    dr = small_sb.tile([128, 1], f32, tag="dr", bufs=8, name="dr")
                    nc.vector.reciprocal(out=dr, in_=d_ap)
                    nc.tensor.matmul(
                        y_ap, lhsT=oT_sb[:, ssl], rhs=woeT_sb,
                        start=True, stop=True,
                    )
                    y_sb = ysb_pool.tile([128, C], f32, tag="y", name="y_sb")
                    nq_row = b * (NQB // 128) + sub
                    # y = y_ps / delta + (x.T + bo_eff)   (one DVE pass)
                    nc.vector.scalar_tensor_tensor(
                        y_sb,
                        y_ap,
                        dr[:, 0:1],
                        xresT_sb[:, nq_row, :],
                        ALU.mult,
                        ALU.add,
                    )
                    eng.dma_start(
                        out=out_d[nq_row * 128 : (nq_row + 1) * 128, :], in_=y_sb
                    )

                return run

            for b, oT_ps, dacc_ps in specs:
                pieces.append(copies(b, oT_ps, dacc_ps))
            qd = 0
            for sub in range(NQB // 128):
                for b, _, _ in specs:
                    pieces.append(sub_piece(b, sub, queues[qd % len(queues)]))
                    qd += 1
            return pieces

        # ---------------------------------------- merged proj+attention loop
        # Blocks 0+1 run while the kT/v2 projections stream in.
        oT0 = ps_acc.tile([128, NQB], f32, tag="oT", name="oT0")
        oT1 = ps_acc.tile([128, NQB], f32, tag="oT", name="oT1")
        dacc0 = ps_acc.tile([128, NQB], f32, tag="dacc", name="dacc0")
        dacc1 = ps_acc.tile([128, NQB], f32, tag="dacc", name="dacc1")
        oTsA, daccsA = (oT0, oT1), (dacc0, dacc1)

        # Warm the PE (HAM un-throttles after ~3.4us of sustained matmul)
        # on weights-only junk while the first x8 slice is still in
        # flight, so the real head runs at 2.4 GHz.
        warm = ps_pair.tile([128, 2, NQB], f32, tag="sp", name="warm")
        for _ in range(10):
            nc.tensor.matmul(
                warm[:, 0, :], lhsT=w8_sb[:, 0, 0, :], rhs=w8_sb[:, 0],
                start=True, stop=True,
            )

        # Phase A: every projection rides a pair's scratch slot - no extra
        # PSUM rotations.  Slot A (first tile): kT block 0 at pair 0, then
        # v2 for pair p-1 (one pair ahead of the lagged PV consumer).
        # Slot B (second tile): qT block 1 at pair 0, kT block j at pair
        # 2j-1, qT blocks 2,3 at pairs 2,4, v2 pair 31 at pair 31.
        def slotA_piece(p):
            if p == 0:
                return lambda sp: k_proj_into(sp, 0)
            j, h = (p - 1) // 2, (p - 1) % 2
            return lambda sp: v_chunks_into(sp, j, h)

        def slotB_piece(p):
            if p == 0:
                return lambda sp: q_proj_into(sp, 1)
            if p == 31:
                return lambda sp: v_chunks_into(sp, 15, 1)
            if p % 2 == 1:
                j2 = (p + 1) // 2
                if j2 < NKCHUNKS // 4:
                    return lambda sp: k_proj_into(sp, j2)
                return None
            if p == 2:
                return lambda sp: q_proj_into(sp, 2)
            if p == 4:
                return lambda sp: q_proj_into(sp, 3)
            return None

        q_proj(0)
        pend = None
        for p in range(NPAIRS):
            pts = emit_pair(p, (0, 1), (slotA_piece(p), slotB_piece(p)))
            if pend is not None:
                pv_dacc(pend[0], pend[1], oTsA, daccsA)
            pend = (p, pts)
        pv_dacc(pend[0], pend[1], oTsA, daccsA)

        # Blocks 2+3: pure attention (ACT-bound), kT/v2 already resident.
        # Blocks 0+1's tails dribble into the PE/DVE slack of this phase.
        oT2 = ps_acc.tile([128, NQB], f32, tag="oT", name="oT2")
        oT3 = ps_acc.tile([128, NQB], f32, tag="oT", name="oT3")
        dacc2 = ps_acc.tile([128, NQB], f32, tag="dacc", name="dacc2")
        dacc3 = ps_acc.tile([128, NQB], f32, tag="dacc", name="dacc3")
        oTsB, daccsB = (oT2, oT3), (dacc2, dacc3)

        tailsA = block_tail_pieces(
            [(0, oT0, dacc0), (1, oT1, dacc1)], [nc.sync, nc.gpsimd]
        )
        pend = None
        ti = 0
        for p in range(NPAIRS):
            pts = emit_pair(p, (2, 3))
            if pend is not None:
                pv_dacc(pend[0], pend[1], oTsB, daccsB)
            pend = (p, pts)
            if p >= 1 and ti < len(tailsA):
                tailsA[ti]()
                ti += 1
        pv_dacc(pend[0], pend[1], oTsB, daccsB)
        while ti < len(tailsA):
            tailsA[ti]()
            ti += 1

        # End tails: nothing left to hide behind.  Use the freed attention
        # accumulator banks instead of the S rotation and spread the store
        # issues over queues whose engines are idle by now.
        for piece in block_tail_pieces(
            [(2, oT2, dacc2), (3, oT3, dacc3)],
            [nc.sync, nc.scalar, nc.gpsimd],
            acc=True,
        ):
            piece()

        for pool in (
            ps_acc,
            ps_pair,
            ysb_pool,
            small_sb,
            pt_pool,
            persist,
            singles,
        ):
            pool.release()

    _split_excess_waits(nc)
    return nc


def _prep_weights(Wq, bq, Wk, bk, Wv, bv, Wo, bo):
    import ml_dtypes

    bf = ml_dtypes.bfloat16
    f8 = ml_dtypes.float8_e4m3fn

    def wT8(Wm):  # [o, C] -> lhsT layout [ci, cio, o], fp8, x16 prescale
        return np.ascontiguousarray(
            (Wm * WSCALE).T.reshape(CO, 128, -1).transpose(1, 0, 2)
        ).astype(f8)

    Wo_eff = Wo.reshape(C, CO, CK).sum(axis=1)            # [C, CK]
    bo_eff = bo + Wo_eff @ bv                             # [C]
    w8 = np.ascontiguousarray(
        np.stack([wT8(Wq), wT8(Wk), wT8(Wv)], axis=1)
    )                                                      # [128, 3, CO, CK]
    return {
        "w8": w8,
        # oT accumulates 16*o; divide back out through the output projection
        "woeT": np.ascontiguousarray(Wo_eff.T / WSCALE).astype(bf),  # [CK, C]
        "bqs": (bq * WSCALE).reshape(128, 1).astype(np.float32),
    }, bo_eff


def kernel(x, Wq, bq, Wk, bk, Wv, bv, Wo, bo):
    import ml_dtypes

    _ensure_axon_hooks_module()
    from concourse.bass_utils import run_bass_kernel_spmd

    f8 = ml_dtypes.float8_e4m3fn
    x = np.asarray(x, dtype=np.float32)
    wmaps, bo_eff = _prep_weights(
        np.asarray(Wq, np.float32),
        np.asarray(bq, np.float32),
        np.asarray(Wk, np.float32),
        np.asarray(bk, np.float32),
        np.asarray(Wv, np.float32),
        np.asarray(bv, np.float32),
        np.asarray(Wo, np.float32),
        np.asarray(bo, np.float32),
    )

    xf = x.reshape(B, C, N)
    x8_b = []
    for b in range(B):
        x8_b.append(
            np.ascontiguousarray(
                xf[b].reshape(CO, 128, N).transpose(1, 0, 2)
            ).astype(f8)
        )
    in_maps = []
    for core in range(NCORES):
        b, s = divmod(core, SEQ_SHARDS)
        # rotate the sequence axis so this core's query chunk sits at 0
        x8 = np.roll(x8_b[b], -s * NCH, axis=2) if s else x8_b[b]
        xchunkT = xf[b][:, s * NCH : (s + 1) * NCH].T  # [NCH, C]
        xresT = np.ascontiguousarray(
            (xchunkT + bo_eff[None, :])
            .reshape(NCH // 128, 128, C)
            .transpose(1, 0, 2)
        ).astype(np.float32)
        in_maps.append({"x8": x8, "xresT": xresT, **wmaps})

    if "nc" not in _cache:
        _cache["nc"] = build_bass()
    res = run_bass_kernel_spmd(_cache["nc"], in_maps, list(range(NCORES)))
    _cache["last_results"] = res

    y = np.empty((B, C, N), dtype=np.float32)
    for core in range(NCORES):
        b, s = divmod(core, SEQ_SHARDS)
        y[b][:, s * NCH : (s + 1) * NCH] = res.results[core]["out"].T
    return y.reshape(B, C, D, H, W)
